# revision 32
# baseline (speedup 1.0000x reference)
"""Expert-parallel MoE (Kimi/DeepSeek-V3 sparse block) on 8 trn2 NeuronCores.

Strategy:
  - Host computes the sigmoid gate + group-limited top-2 routing in float64
    and gathers each expert's tokens into a transposed, k-interleaved batch.
  - Capacity R = align8(max expert load) is chosen from the *actual*
    routing, so per-core work is R + 2*512 shared token-instances instead of
    a fixed 1536 + 1024 — the program is compiled per capacity and cached.
  - Core e runs expert e's FFN over its R-token batch plus the full shared
    expert (SH=1024, one pass) over token slice [512e : 512(e+1)].
  - All matmul operands and the outputs are bf16 (fp32 PSUM accumulate):
    halves HBM<->SBUF traffic; rel-err ~4.6e-3 vs the 2e-2 gate.
  - Every DMA moves >=2KB contiguous per partition (the DMA-efficiency
    knee): the first chunk's weights+x are host-packed into one interleaved
    tensor streamed at k-pair granularity; later sets move whole-matrix.
  - Chunk order E0, SH, E1, ..., Er: shared weights arrive during E0; the
    small expert remainder lands last so the post-matmul tail is short.
  - Up-phase mf-outer (k-outer for the streaming first chunk), down-phase
    kf-outer in two md-halves with per-half output DMAs (outputs flush
    while the second half computes); PSUM->SBUF evacuation alternates
    vector/scalar; the final DMAs use both gpsimd and scalar queues.
  - N=512 warm-up matmuls on a zeroed tile run during the initial DMA wait
    so the PE's HAM clock gate is already 8/8 when real work starts.
"""

from contextlib import ExitStack

import numpy as np
import ml_dtypes

import concourse.bacc as bacc
import concourse.tile as tile
import concourse.mybir as mybir
from concourse import bass_utils

# --- model dims (hardcoded per problem spec) ---
B, S, D = 2, 2048, 1024
T = B * S                 # 4096 tokens
E, F = 8, 512             # routed experts / expert intermediate
SH = 1024                 # shared intermediate
TOP_K, N_GROUP, TOPK_GROUP = 2, 4, 2
SCALE = 2.5

N_CORES = 8
P = 128                   # SBUF partitions
KD = D // P               # 8 contraction tiles over D
KF = F // P               # 4 F-tiles per expert
KS = SH // P              # 8 F-tiles for the shared expert
NT = 512                  # max matmul moving free dim (one PSUM bank fp32)
SHT = T // N_CORES        # 512 shared-expert tokens per core
R_MAX = 4096              # capacity ceiling (SBUF); overflow -> host
N_WARM = 8                # N=512 HAM warm-up matmuls (~3.4us busy)

F32 = mybir.dt.float32
BF16 = mybir.dt.bfloat16
BF16_NP = ml_dtypes.bfloat16

_CACHE: dict = {}


def _chunk_plan(R):
    """Chunks: (mode, col_off, ncols); mode 'e' (expert) or 'sh' (shared).

    Order: first expert chunk (k-pair DMA streaming), then the one-pass
    shared chunk, then remaining expert chunks — the small expert remainder
    lands last so the post-matmul tail (copies + output DMA) is short."""
    echunks = []
    c = 0
    while c < R:
        n = min(NT, R - c)
        echunks.append(("e", c, n))
        c += n
    schunks = []
    c = 0
    while c < SHT:
        n = min(NT, SHT - c)
        schunks.append(("sh", c, n))
        c += n
    return tuple(echunks[:1] + schunks + echunks[1:])


def _emit(nc, R):
    chunks = _chunk_plan(R)
    n0 = chunks[0][2]  # first chunk size
    W0 = 2 * F + n0    # packed set0 row: w1 | w3 | x0 per k

    s0d = nc.dram_tensor("s0pack", [P, KD, W0], BF16, kind="ExternalInput").ap()
    xshd = nc.dram_tensor("xshd", [P, KD, SHT], BF16, kind="ExternalInput").ap()
    xrd = (nc.dram_tensor("xrd", [P, KD, R - n0], BF16, kind="ExternalInput").ap()
           if R > n0 else None)
    w1shd = nc.dram_tensor("w1_sh", [P, KD, SH], BF16, kind="ExternalInput").ap()
    w3shd = nc.dram_tensor("w3_sh", [P, KD, SH], BF16, kind="ExternalInput").ap()
    w2d = [nc.dram_tensor("w2_0", [P, KF, D], BF16, kind="ExternalInput").ap(),
           nc.dram_tensor("w2_sh", [P, KS, D], BF16, kind="ExternalInput").ap()]
    youts = [nc.dram_tensor(f"y{j}", [P, KD, n], BF16, kind="ExternalOutput").ap()
             for j, (mode, c0, n) in enumerate(chunks)]

    silu = mybir.ActivationFunctionType.Silu

    with tile.TileContext(nc) as tc, ExitStack() as ctx:
        wpool = ctx.enter_context(tc.tile_pool(name="wpool", bufs=1))
        xpool = ctx.enter_context(tc.tile_pool(name="xpool", bufs=1))
        hpool = ctx.enter_context(tc.tile_pool(name="hpool", bufs=2))
        opool = ctx.enter_context(tc.tile_pool(name="opool", bufs=2))
        pspool = ctx.enter_context(tc.tile_pool(name="pspool", bufs=1, space="PSUM"))

        ps_tags = [f"ps{i}" for i in range(8)]

        # ---- SBUF weight tiles: set0 packed at k-pair granularity
        s0p = [wpool.tile([P, 2, W0], BF16, name=f"s0p{j}") for j in range(KD // 2)]
        w2t = [wpool.tile([P, KF, D], BF16, name="w2t0"),
               wpool.tile([P, KS, D], BF16, name="w2tsh")]
        w1sh = wpool.tile([P, KD, SH], BF16, name="w1sh")
        w3sh = wpool.tile([P, KD, SH], BF16, name="w3sh")

        xsh = xpool.tile([P, KD, SHT], BF16, name="xsh")
        xrest = (xpool.tile([P, KD, R - n0], BF16, name="xrest")
                 if R > n0 else None)

        # ---- HAM warm-up: N=512 matmuls on a zeroed tile during DMA wait
        wu = xpool.tile([P, NT], BF16, name="wu")
        nc.vector.memset(wu[:], 0)
        wups = pspool.tile([P, NT], F32, name="wups", tag=ps_tags[7])
        for _ in range(N_WARM):
            nc.tensor.matmul(wups[0:64, :], wu[:, 0:64], wu[:],
                             start=True, stop=True)

        # ---- input DMA stream (sync queue), in consumption order
        # k=0 split w1|x|w3 so the very first h1 matmuls start sooner
        nc.sync.dma_start(s0p[0][:, 0, 0:F], s0d[:, 0, 0:F])
        nc.sync.dma_start(s0p[0][:, 0, 2 * F:W0], s0d[:, 0, 2 * F:W0])
        nc.sync.dma_start(s0p[0][:, 0, F:2 * F], s0d[:, 0, F:2 * F])
        nc.sync.dma_start(s0p[0][:, 1, :], s0d[:, 1, :])
        for j in range(1, KD // 2):
            nc.sync.dma_start(s0p[j][:], s0d[:, 2 * j:2 * j + 2, :])
        nc.sync.dma_start(w2t[0][:], w2d[0][:])
        nc.sync.dma_start(w1sh[:], w1shd[:])
        nc.sync.dma_start(xsh[:], xshd[:])
        nc.sync.dma_start(w3sh[:], w3shd[:])
        nc.sync.dma_start(w2t[1][:], w2d[1][:])
        if xrest is not None:
            nc.sync.dma_start(xrest[:], xrd[:])

        def wsl(which, mode, k, blk):
            if mode == "e":
                off = 0 if which == 1 else F
                return s0p[k // 2][:, k % 2, off + blk * P:off + (blk + 1) * P]
            t = w1sh if which == 1 else w3sh
            return t[:, k, blk * P:(blk + 1) * P]

        def xsrc(ci, mode, k, c0, n):
            if mode == "sh":
                return xsh[:, k, c0:c0 + n]
            if ci == 0:
                return s0p[k // 2][:, k % 2, 2 * F:2 * F + n]
            return xrest[:, k, c0 - n0:c0 - n0 + n]

        last_ci = len(chunks) - 1

        def ffn(ci, mode, c0, n):
            yj = youts[ci]
            kfs = KF if mode == "e" else KS
            w2 = w2t[0 if mode == "e" else 1]
            h1s, h3s, hts = [None] * kfs, [None] * kfs, [None] * kfs

            def act_mul(mf):
                a = hpool.tile([P, NT], F32, name="asb", tag="silu")
                nc.scalar.activation(a[:, 0:n], h1s[mf][:, 0:n], silu)
                ht = hpool.tile([P, NT], BF16, name="htsb", tag=f"ht{mf}")
                nc.vector.tensor_mul(ht[:, 0:n], a[:, 0:n], h3s[mf][:, 0:n])
                hts[mf] = ht

            if ci == 0:
                # k-outer: consume weight/x k-pairs in DMA arrival order
                for mf in range(kfs):
                    h1s[mf] = pspool.tile([P, NT], F32, name="h1ps", tag=ps_tags[2 * mf])
                    h3s[mf] = pspool.tile([P, NT], F32, name="h3ps", tag=ps_tags[2 * mf + 1])
                for k in range(KD):
                    xs = xsrc(ci, mode, k, c0, n)
                    st, sp = (k == 0), (k == KD - 1)
                    for mf in range(kfs):
                        nc.tensor.matmul(h1s[mf][:, 0:n], wsl(1, mode, k, mf),
                                         xs, start=st, stop=sp)
                    for mf in range(kfs):
                        nc.tensor.matmul(h3s[mf][:, 0:n], wsl(3, mode, k, mf),
                                         xs, start=st, stop=sp)
                for mf in range(kfs):
                    act_mul(mf)
            else:
                # mf-outer: act/mul of pair mf overlaps matmuls of pair mf+1
                for mf in range(kfs):
                    h1s[mf] = pspool.tile([P, NT], F32, name="h1ps",
                                          tag=ps_tags[(2 * mf) % 8])
                    for k in range(KD):
                        nc.tensor.matmul(h1s[mf][:, 0:n], wsl(1, mode, k, mf),
                                         xsrc(ci, mode, k, c0, n),
                                         start=(k == 0), stop=(k == KD - 1))
                    h3s[mf] = pspool.tile([P, NT], F32, name="h3ps",
                                          tag=ps_tags[(2 * mf + 1) % 8])
                    for k in range(KD):
                        nc.tensor.matmul(h3s[mf][:, 0:n], wsl(3, mode, k, mf),
                                         xsrc(ci, mode, k, c0, n),
                                         start=(k == 0), stop=(k == KD - 1))
                    act_mul(mf)

            # down-phase in two md-halves (kf-outer inside each) so the
            # first half's outputs flush while the second half computes
            copy_eng = [nc.vector.tensor_copy, nc.scalar.copy]
            ysb = opool.tile([P, KD, NT], BF16, name="ysb", tag="ysb")
            H = KD // 2
            for half in range(2):
                mds = range(half * H, (half + 1) * H)
                yps = {md: pspool.tile([P, NT], F32, name="yps", tag=ps_tags[md])
                       for md in mds}
                for kf in range(kfs):
                    st, sp = (kf == 0), (kf == kfs - 1)
                    for md in mds:
                        nc.tensor.matmul(yps[md][:, 0:n],
                                         w2[:, kf, md * P:(md + 1) * P],
                                         hts[kf][:, 0:n], start=st, stop=sp)
                hsl = slice(half * H, (half + 1) * H)
                for md in mds:
                    copy_eng[md % 2](ysb[:, md, 0:n], yps[md][:, 0:n])
                if ci == last_ci:
                    # quarter DMAs on alternating queues: tiny final flush
                    for q in range(2):
                        qsl = slice(half * H + q * 2, half * H + (q + 1) * 2)
                        eng = nc.gpsimd if (half * 2 + q) % 2 == 0 else nc.scalar
                        eng.dma_start(yj[:, qsl, :], ysb[:, qsl, 0:n])
                else:
                    dma_eng = nc.gpsimd if half == 0 else nc.scalar
                    dma_eng.dma_start(yj[:, hsl, :], ysb[:, hsl, 0:n])

        for ci, (mode, c0, n) in enumerate(chunks):
            ffn(ci, mode, c0, n)


def _get_nc(R):
    key = ("nc", R)
    if key not in _CACHE:
        nc = bacc.Bacc("TRN2", target_bir_lowering=False, debug=False,
                       num_devices=N_CORES)
        _emit(nc, R)
        nc.compile()
        _CACHE[key] = nc
    return _CACHE[key]


def _gate_numpy(x2d, gate_w, gate_bias):
    """Replicates reference _moe_gate in float64 (routing-stable)."""
    xl = x2d.astype(np.float64)
    logits = xl @ gate_w.astype(np.float64).T
    scores = 1.0 / (1.0 + np.exp(-logits))
    sc = scores + gate_bias.astype(np.float64)[None, :]
    grp = sc.reshape(T, N_GROUP, E // N_GROUP)
    group_scores = np.sort(grp, axis=-1)[:, :, -2:].sum(-1)
    gidx = np.argsort(-group_scores, axis=-1, kind="stable")[:, :TOPK_GROUP]
    gmask = np.zeros((T, N_GROUP), bool)
    gmask[np.arange(T)[:, None], gidx] = True
    smask = np.repeat(gmask, E // N_GROUP, axis=1)
    tmp = np.where(smask, sc, 0.0)
    tidx = np.argsort(-tmp, axis=-1, kind="stable")[:, :TOP_K]
    tw = np.take_along_axis(scores, tidx, axis=1)
    tw = tw / (tw.sum(-1, keepdims=True) + 1e-20)
    return tidx, (tw * SCALE).astype(np.float32)


def _ffn_host(x, w1e, w2e, w3e):
    """Host fallback for capacity-overflow tokens (pathological skew only)."""
    h = x @ w1e.T
    h = (h / (1.0 + np.exp(-h))) * (x @ w3e.T)
    return h @ w2e.T


def _ikp(mat, kt):
    """[kt*P, X] -> [P, kt, X] bf16 (k-interleaved, partition-major)."""
    return np.asarray(mat).reshape(kt, P, -1).transpose(1, 0, 2).astype(BF16_NP)


def kernel(hidden_states, gate_w, gate_bias, w1, w2, w3,
           shared_gate_w, shared_up_w, shared_down_w):
    hidden_states = np.ascontiguousarray(np.asarray(hidden_states, np.float32))
    gate_w = np.asarray(gate_w, np.float32)
    gate_bias = np.asarray(gate_bias, np.float32)
    w1 = np.asarray(w1, np.float32)
    w2 = np.asarray(w2, np.float32)
    w3 = np.asarray(w3, np.float32)
    shared_gate_w = np.asarray(shared_gate_w, np.float32)
    shared_up_w = np.asarray(shared_up_w, np.float32)
    shared_down_w = np.asarray(shared_down_w, np.float32)

    x2d = hidden_states.reshape(T, D)
    tidx, tw = _gate_numpy(x2d, gate_w, gate_bias)

    counts = np.bincount(tidx.ravel(), minlength=E)
    R = int(min(-(-counts.max() // 8) * 8, R_MAX))
    R = max(R, 8)
    chunks = _chunk_plan(R)
    n0 = chunks[0][2]

    w1sh_i = _ikp(shared_gate_w.T, KD)
    w3sh_i = _ikp(shared_up_w.T, KD)
    w2sh_i = _ikp(shared_down_w.T, KS)

    x2dT = np.ascontiguousarray(x2d.T)  # [D, T]
    in_maps = []
    idx_list, wt_list, n_list, overflow = [], [], [], []
    for e in range(E):
        rows, slots = np.nonzero(tidx == e)
        n = len(rows)
        if n > R:
            overflow.append((e, rows[R:], slots[R:]))
            rows, slots = rows[:R], slots[:R]
            n = R
        idx_list.append(rows)
        wt_list.append(tw[rows, slots])
        n_list.append(n)
        xe = np.zeros((D, R), np.float32)
        xe[:, :n] = x2dT[:, rows]
        im = {
            "s0pack": np.concatenate([_ikp(w1[e].T, KD), _ikp(w3[e].T, KD),
                                      _ikp(xe[:, :n0], KD)], axis=2),
            "xshd": _ikp(x2dT[:, e * SHT:(e + 1) * SHT], KD),
            "w2_0": _ikp(w2[e].T, KF),
            "w1_sh": w1sh_i, "w3_sh": w3sh_i, "w2_sh": w2sh_i,
        }
        if R > n0:
            im["xrd"] = _ikp(xe[:, n0:], KD)
        in_maps.append(im)

    nc = _get_nc(R)
    res = bass_utils.run_bass_kernel_spmd(
        nc, in_maps, core_ids=list(range(N_CORES))
    )
    _CACHE["last_res"] = res

    y = np.zeros((T, D), np.float32)
    for e in range(E):
        n = n_list[e]
        rows = idx_list[e]
        wts = wt_list[e]
        out = res.results[e]
        for j, (mode, c0, nj) in enumerate(chunks):
            # y{j} is [P, KD, nj]; row d = md*P + p
            blk = np.asarray(out[f"y{j}"], np.float32).transpose(1, 0, 2).reshape(D, nj)
            if mode == "e":
                lo, hi = c0, min(c0 + nj, n)
                if hi > lo:
                    y[rows[lo:hi]] += wts[lo:hi, None] * blk[:, 0:hi - lo].T
            else:  # shared output for token slice
                sl = slice(e * SHT + c0, e * SHT + c0 + nj)
                y[sl] += blk.T
    for e, rows, slots in overflow:
        y[rows] += tw[rows, slots][:, None] * _ffn_host(x2d[rows], w1[e], w2[e], w3[e])

    return y.reshape(B, S, D)
